# revision 5
# baseline (speedup 1.0000x reference)
"""Causal attention layer (RoPE + QK-RMSNorm + value-residual) on 8 trn2 cores.

Sharding: pure data/tensor parallel with zero collectives. Core c handles
batch b = c//2 and head-group hg = c%2 (6 of 12 heads). Each core computes
q/k/v projections for its heads over the full sequence (it receives the whole
x[b]), runs causal attention for its 6 heads, and produces a partial
out-projection [T, D] (sum over its heads' Wo rows). Host sums the two
partials per batch.

Per-core kernel layout choices:
  - projections run on PE in f32r (fp22, full PE rate at N>=256)
  - RMS stats + RoPE on DVE in the natural [t, d] layout, then PE-transpose
    q,k to [d, t] fp16 for attention
  - attention scores ST[j, i] = k_j . q_i so softmax denominators fold into
    the PV matmul (ones column appended to v) and k's RMS scale folds into
    the exp's per-partition scale
  - PV runs in fp16 with the lower-triangle mask applied only to diagonal
    128x128 blocks; strictly-upper j-tiles are skipped entirely
"""

import sys

sys.path.insert(0, "/opt/trn_rl_repo")

import numpy as np

import concourse.bass as bass
import concourse.mybir as mybir
import concourse.tile as tile
from concourse import bacc
from concourse.masks import make_identity

F32 = mybir.dt.float32
F32R = mybir.dt.float32r
F16 = mybir.dt.float16
AX = mybir.AxisListType

B, T, D, H = 4, 2048, 768, 12
HD = 64
NCORES = 8
HPC = 6          # heads per core
DC = HPC * HD    # 384
NT = T // 128    # 16 t-tiles
KT = D // 128    # 6 contraction tiles
DB = DC // 128   # 3 d-blocks


def build_bass():
    nc = bacc.Bacc("TRN2")
    xT_d = nc.dram_tensor("xT", [D, T], F32R, kind="ExternalInput")
    wq_d = nc.dram_tensor("wq", [D, DC], F32R, kind="ExternalInput")
    wk_d = nc.dram_tensor("wk", [D, DC], F32R, kind="ExternalInput")
    wv_d = nc.dram_tensor("wv", [D, DC], F32R, kind="ExternalInput")
    wo_d = nc.dram_tensor("wo", [DC, D], F16, kind="ExternalInput")
    v1p_d = nc.dram_tensor("v1p", [T, DC], F32, kind="ExternalInput")
    cosf_d = nc.dram_tensor("cosf", [T, DC], F32, kind="ExternalInput")
    sinf_d = nc.dram_tensor("sinf", [T, DC], F32, kind="ExternalInput")
    tri_d = nc.dram_tensor("tri", [128, 128], F16, kind="ExternalInput")
    out_d = nc.dram_tensor("out", [T, D], F32, kind="ExternalOutput")

    with tile.TileContext(nc) as tc:
        with tc.tile_pool(name="persist", bufs=1) as P:
            ident = P.tile([128, 128], F16)
            make_identity(nc, ident)
            tri = P.tile([128, 128], F16)
            nc.sync.dma_start(out=tri, in_=tri_d[:, :])
            recipk = P.tile([128, NT * HPC], F32)
            epsq = P.tile([128, 1], F32)
            nc.vector.memset(epsq, 1e-8)
            epsk = P.tile([128, 1], F32)
            nc.vector.memset(epsk, HD * 1e-8)
            wo_sb = [P.tile([128, D], F16, tag=f"wo{k}", name=f"wo{k}") for k in range(DB)]
            for k in range(DB):
                nc.sync.dma_start(out=wo_sb[k], in_=wo_d[k * 128:(k + 1) * 128, :])
            v_sb = [P.tile([128, HPC, HD + 1], F16, tag=f"v{i}", name=f"v{i}") for i in range(NT)]
            qT_sb = [P.tile([128, T], F16, tag=f"qT{k}", name=f"qT{k}") for k in range(DB)]
            kT_sb = [P.tile([128, T], F16, tag=f"kT{k}", name=f"kT{k}") for k in range(DB)]
            yT_sb = [P.tile([128, T], F16, tag=f"yT{k}", name=f"yT{k}") for k in range(DB)]

            # ---------------- phase 1: projections + rms + rope ----------
            with tc.tile_pool(name="p1sb", bufs=1) as P1, \
                 tc.tile_pool(name="p1dma", bufs=3) as PD, \
                 tc.tile_pool(name="p1tmp", bufs=2) as PT_, \
                 tc.tile_pool(name="p1sm", bufs=8) as PS, \
                 tc.tile_pool(name="p1ro", bufs=1) as PR, \
                 tc.tile_pool(name="p1ps", bufs=2, space="PSUM") as PP, \
                 tc.tile_pool(name="p1tp", bufs=2, space="PSUM") as PTP:
                xT_sb = [P1.tile([128, T], F32R, tag=f"xT{k}", name=f"xT{k}") for k in range(KT)]
                w_sb = {}
                for nm, dram in (("wq", wq_d), ("wk", wk_d), ("wv", wv_d)):
                    w_sb[nm] = [P1.tile([128, DC], F32R, tag=f"{nm}{k}", name=f"{nm}{k}")
                                for k in range(KT)]
                    for k in range(KT):
                        nc.sync.dma_start(out=w_sb[nm][k],
                                          in_=dram[k * 128:(k + 1) * 128, :])
                for k in range(KT):
                    nc.sync.dma_start(out=xT_sb[k],
                                      in_=xT_d[k * 128:(k + 1) * 128, :])

                for g in range(NT // 4):
                    ro_tiles = {"q": [], "k": []}
                    for i in range(4):
                        tt = 4 * g + i
                        ts = slice(tt * 128, (tt + 1) * 128)
                        cos_t = PD.tile([128, DC], F32, tag="cos")
                        nc.sync.dma_start(out=cos_t, in_=cosf_d[ts, :])
                        sin_t = PD.tile([128, DC], F32, tag="sin")
                        nc.sync.dma_start(out=sin_t, in_=sinf_d[ts, :])

                        # v projection + residual mix
                        vps = PP.tile([128, DC], F32, tag="vps")
                        for k in range(KT):
                            nc.tensor.matmul(vps, lhsT=xT_sb[k][:, ts],
                                             rhs=w_sb["wv"][k],
                                             start=(k == 0), stop=(k == KT - 1))
                        v1t = PD.tile([128, DC], F32, tag="v1")
                        nc.sync.dma_start(out=v1t, in_=v1p_d[ts, :])
                        vt = v_sb[tt]
                        nc.vector.memset(vt[:, :, HD:HD + 1], 1.0)
                        nc.vector.tensor_add(
                            vt[:, :, 0:HD],
                            vps.rearrange("p (h d) -> p h d", h=HPC),
                            v1t.rearrange("p (h d) -> p h d", h=HPC))

                        for nm in ("q", "k"):
                            ps = PP.tile([128, DC], F32, tag=f"{nm}ps")
                            for k in range(KT):
                                nc.tensor.matmul(ps, lhsT=xT_sb[k][:, ts],
                                                 rhs=w_sb["w" + nm][k],
                                                 start=(k == 0), stop=(k == KT - 1))
                            # sum of squares per head (pre-rope == post-rope)
                            sq = PT_.tile([128, DC], F32, tag="sq")
                            nc.scalar.square(sq, ps)
                            ssq = PS.tile([128, HPC], F32, tag="ssq")
                            nc.vector.tensor_reduce(
                                out=ssq, in_=sq.rearrange("p (h d) -> p h d", h=HPC),
                                axis=AX.X, op=mybir.AluOpType.add)
                            rms = PS.tile([128, HPC], F32, tag="rms")
                            if nm == "q":
                                # 1/sqrt(ssq/64 + eps)
                                nc.scalar.activation(
                                    out=rms, in_=ssq,
                                    func=mybir.ActivationFunctionType.Sqrt,
                                    scale=1.0 / HD, bias=epsq)
                                rec = PS.tile([128, HPC], F32, tag="rec")
                                nc.vector.reciprocal(out=rec, in_=rms)
                            else:
                                # 0.125/sqrt(ssq/64 + eps) == 1/sqrt(ssq + 64e-8)
                                nc.scalar.activation(
                                    out=rms, in_=ssq,
                                    func=mybir.ActivationFunctionType.Sqrt,
                                    scale=1.0, bias=epsk)
                                nc.vector.reciprocal(
                                    out=recipk[:, tt * HPC:(tt + 1) * HPC], in_=rms)
                            # rope: y1 = x1*c + x2*s ; y2 = x2*c - x1*s
                            mc = PT_.tile([128, DC], F32, tag="mc")
                            nc.vector.tensor_mul(mc, ps, cos_t)
                            ms = PT_.tile([128, DC], F32, tag="ms")
                            nc.vector.tensor_mul(ms, ps, sin_t)
                            ro = PR.tile([128, DC], F16, tag=f"{nm}ro{i}")
                            ro4 = ro.rearrange("p (h s d) -> p h s d", h=HPC, s=2)
                            mc4 = mc.rearrange("p (h s d) -> p h s d", h=HPC, s=2)
                            ms4 = ms.rearrange("p (h s d) -> p h s d", h=HPC, s=2)
                            nc.vector.tensor_add(ro4[:, :, 0, :], mc4[:, :, 0, :],
                                                 ms4[:, :, 1, :])
                            nc.vector.tensor_sub(ro4[:, :, 1, :], mc4[:, :, 1, :],
                                                 ms4[:, :, 0, :])
                            if nm == "q":
                                ro3 = ro.rearrange("p (h d) -> p h d", h=HPC)
                                for h in range(HPC):
                                    nc.vector.tensor_scalar_mul(
                                        out=ro3[:, h, :], in0=ro3[:, h, :],
                                        scalar1=rec[:, h:h + 1])
                            ro_tiles[nm].append(ro)
                    # transpose the 4 prepared t-tiles into qT/kT
                    for nm, dst in (("q", qT_sb), ("k", kT_sb)):
                        for db in range(DB):
                            tp = PTP.tile([128, 512], F16, tag="tp")
                            for ii in range(4):
                                nc.tensor.transpose(
                                    tp[:, ii * 128:(ii + 1) * 128],
                                    ro_tiles[nm][ii][:, db * 128:(db + 1) * 128],
                                    ident)
                            nc.vector.tensor_copy(
                                out=dst[db][:, g * 512:(g + 1) * 512], in_=tp)

            # ---------------- phase 2: attention -------------------------
            with tc.tile_pool(name="p2pt", bufs=16) as PPT, \
                 tc.tile_pool(name="p2sm", bufs=4) as P2S, \
                 tc.tile_pool(name="p2yh", bufs=4) as P2Y, \
                 tc.tile_pool(name="p2st", bufs=2, space="PSUM") as PST, \
                 tc.tile_pool(name="p2ya", bufs=1, space="PSUM") as PYA, \
                 tc.tile_pool(name="p2yt", bufs=2, space="PSUM") as PYT:
                for hp in range(DB):  # head pair block
                    for c in range(4):  # i-chunk of 512
                        cs = slice(c * 512, (c + 1) * 512)
                        pts = {0: [], 1: []}
                        for jt in range(4 * c + 4):
                            js = slice(jt * 128, (jt + 1) * 128)
                            r = jt - 4 * c
                            for half in (0, 1):
                                hloc = 2 * hp + half
                                rows = slice(half * 64, (half + 1) * 64)
                                st = PST.tile([128, 512], F32, tag=f"st{half}")
                                nc.tensor.matmul(
                                    st, lhsT=kT_sb[hp][rows, js],
                                    rhs=qT_sb[hp][rows, cs],
                                    start=True, stop=True,
                                    tile_position=(half * 64, 0))
                                pt = PPT.tile([128, 512], F16, tag=f"pt{half}")
                                c0 = 128 * r if r > 0 else 0
                                nc.scalar.activation(
                                    out=pt[:, c0:512], in_=st[:, c0:512],
                                    func=mybir.ActivationFunctionType.Exp,
                                    scale=recipk[:, jt * HPC + hloc:
                                                 jt * HPC + hloc + 1])
                                if r >= 0:
                                    nc.vector.tensor_mul(
                                        pt[:, 128 * r:128 * (r + 1)],
                                        pt[:, 128 * r:128 * (r + 1)], tri)
                                pts[half].append(pt)
                        for half in (0, 1):
                            hloc = 2 * hp + half
                            ya = PYA.tile([128, 4 * (HD + 1)], F32, tag=f"ya{half}")
                            for il in range(4):
                                it = 4 * c + il
                                ysl = slice(il * 65, il * 65 + 65)
                                for jt in range(it + 1):
                                    nc.tensor.matmul(
                                        ya[:, ysl],
                                        lhsT=pts[half][jt][:, il * 128:(il + 1) * 128],
                                        rhs=v_sb[jt][:, hloc, :],
                                        start=(jt == 0), stop=(jt == it))
                            rc = P2S.tile([128, 4], F32, tag="rc")
                            nc.vector.reciprocal(
                                out=rc,
                                in_=ya.rearrange("p (i s) -> p i s", s=65)[:, :, 64])
                            ytp = PYT.tile([64, 512], F16, tag="ytp")
                            for il in range(4):
                                yh = P2Y.tile([128, HD], F16, tag="yh")
                                nc.vector.tensor_scalar_mul(
                                    out=yh, in0=ya[:, il * 65:il * 65 + 64],
                                    scalar1=rc[:, il:il + 1])
                                nc.tensor.transpose(
                                    ytp[:, il * 128:(il + 1) * 128], yh, ident)
                            nc.vector.tensor_copy(
                                out=yT_sb[hp][half * 64:(half + 1) * 64, cs],
                                in_=ytp)

            # ---------------- phase 3: out projection --------------------
            with tc.tile_pool(name="p3ev", bufs=3) as P3E, \
                 tc.tile_pool(name="p3ps", bufs=2, space="PSUM") as P3P:
                for tt in range(NT):
                    ts = slice(tt * 128, (tt + 1) * 128)
                    ops = P3P.tile([128, D], F32, tag="ops")
                    for n0, nw in ((0, 512), (512, 256)):
                        for k in range(DB):
                            nc.tensor.matmul(
                                ops[:, n0:n0 + nw], lhsT=yT_sb[k][:, ts],
                                rhs=wo_sb[k][:, n0:n0 + nw],
                                start=(k == 0), stop=(k == DB - 1))
                    oev = P3E.tile([128, D], F32, tag="oev")
                    nc.vector.tensor_copy(out=oev, in_=ops)
                    nc.sync.dma_start(out=out_d[ts, :], in_=oev)

    nc.compile()
    return nc


_NC_CACHE = None


def _get_nc():
    global _NC_CACHE
    if _NC_CACHE is None:
        _NC_CACHE = build_bass()
    return _NC_CACHE


def make_in_maps(x, cos, sin, v1, Wq, Wk, Wv, Wo, lamb1, lamb2):
    x = np.asarray(x, np.float32)
    cos = np.asarray(cos, np.float32)[0]   # [T, 32]
    sin = np.asarray(sin, np.float32)[0]
    v1 = np.asarray(v1, np.float32)
    Wq = np.asarray(Wq, np.float32)
    Wk = np.asarray(Wk, np.float32)
    Wv = np.asarray(Wv, np.float32)
    Wo = np.asarray(Wo, np.float32)
    l1 = np.float32(np.asarray(lamb1))
    l2 = np.float32(np.asarray(lamb2))

    cosf = np.ascontiguousarray(
        np.tile(np.concatenate([cos, cos], axis=1), (1, HPC)))  # [T, 384]
    sinf = np.ascontiguousarray(
        np.tile(np.concatenate([sin, sin], axis=1), (1, HPC)))
    tri = np.asarray(
        np.arange(128)[None, :] >= np.arange(128)[:, None], np.float16)
    xTs = [np.ascontiguousarray(x[b].T) for b in range(B)]

    in_maps = []
    for c in range(NCORES):
        b, hg = c // 2, c % 2
        colsl = slice(hg * DC, (hg + 1) * DC)
        v1p = np.ascontiguousarray(
            (l2 * v1[b, hg * HPC:(hg + 1) * HPC]).transpose(1, 0, 2)
            .reshape(T, DC))
        in_maps.append({
            "xT": xTs[b],
            "wq": np.ascontiguousarray(Wq[:, colsl]),
            "wk": np.ascontiguousarray(Wk[:, colsl]),
            "wv": np.ascontiguousarray(l1 * Wv[:, colsl]),
            "wo": np.ascontiguousarray(Wo[colsl, :]).astype(np.float16),
            "v1p": v1p,
            "cosf": cosf,
            "sinf": sinf,
            "tri": tri,
        })
    return in_maps


def kernel(x, cos, sin, v1, Wq, Wk, Wv, Wo, lamb1, lamb2):
    from concourse.bass_utils import run_bass_kernel_spmd

    nc = _get_nc()
    in_maps = make_in_maps(x, cos, sin, v1, Wq, Wk, Wv, Wo, lamb1, lamb2)
    res = run_bass_kernel_spmd(nc, in_maps, list(range(NCORES)))
    out = np.empty((B, T, D), np.float32)
    for b in range(B):
        out[b] = res.results[2 * b]["out"] + res.results[2 * b + 1]["out"]
    return out
